# revision 10
# baseline (speedup 1.0000x reference)
"""GRU decoder (nn_Decoder2) Trainium2 Bass kernel — block-diagonal edition.

Per core (pure batch data-parallel over 8 cores): 4096 rows as 2 supergroups
of 4 chunks x 512.  State h lives STACKED: h_s[g] [128, 512] fp16 with
partitions = 4 chunks x 32 hidden units.

Key idea vs the 572us baseline: instead of 32 small M=32 matmuls per step
(one per gate x chunk, each streaming its own 512 rhs columns), each gate is
ONE K=128, M=128, N=512 matmul whose weights are the 32x32 recurrent block
replicated on the diagonal (block (ci,ci) = R'[32,32] at cols 32ci).  The
matmul then reads the stacked state tile DIRECTLY as rhs and produces the
gate for all 4 chunks in a single 512-column stream:
    out[32c+k, b] = sum_h R'[h,k] * h_s[32c+h, b].
The feature/bias/y0 side is a second accumulating matmul with block-diagonal
[72,128] weights (4 x 18 rows: 16 feat + ones + y0) reading a prefetched
feat tile [72, 512].  8 gate-MM pairs + 2 dense MMs per step, ~240ns each
(measured: full-M matmuls stream warm at ~N/2.4GHz with LDWEIGHTS hidden),
vs ~610ns cold-issue per baseline MM.  This also deletes the per-step
h'-scatter DMAs (matmul reads h_s in place).

PSUM: per sg, zr [128,1024] (z|r banks) and xm [128,1024] (xh|mh) = 8 banks.
Elementwise per sg, all ops [128, 512+] wide, fp16:
    zrs = sig(zr)  [128,1024];  t2 = zrs_r * mh;  t3 = t2 + xh;
    hh = tanh(t3);  d = h - hh;  p = zrs_z * d;  h' = hh + p.
Dense(1) folded into recurrent weights host-side (as baseline), so the
recurrence never waits on the dense output; per-step dense is a K=128 block
diag matmul into the freed xh bank rows 0:4, ACT-copied into a 4-step
accumulator and DMA'd out once per 4 steps.
"""
import numpy as np

B, T, F, H = 32768, 48, 16, 32
NCORES = 8
BS = B // NCORES            # 4096 batch per core
CK = 512                    # chunk batch size
NSG = 2                     # supergroups
SGC = 4                     # chunks per supergroup
FR = 18                     # feat rows per chunk: 16 feat + ones + y0

_CACHE = {}


def _prep_weights_f64(kernel, recurrent_kernel, bias_x, bias_h, dense_w, dense_b):
    """Baseline fold (float64): returns wfh [50,128], w0 [50,128], wd4 [128,4].

    Gate column blocks: 0:32 z, 32:64 r, 64:96 xh, 96:128 mh.
    Row layout: 0:32 h, 32:48 feat, 48 ones/bias, 49 y0 (zero in wfh).
    """
    kd = kernel.astype(np.float64)
    rkd = recurrent_kernel.astype(np.float64)
    bxd = bias_x.astype(np.float64)
    bhd = bias_h.astype(np.float64)
    dwd = dense_w.astype(np.float64)[:, 0]
    dbd = float(dense_b.astype(np.float64)[0])

    k0 = kd[0]
    kf = kd[1:]
    dwk0 = np.outer(dwd, k0)

    wfh = np.zeros((50, 128), np.float64)
    w0 = np.zeros((50, 128), np.float64)
    for gi, (lo, hi) in enumerate([(0, 32), (32, 64), (64, 96), (96, 128)]):
        src = [slice(0, 32), slice(32, 64), slice(64, 96), slice(64, 96)][gi]
        if gi < 3:
            wfh[32:48, lo:hi] = kf[:, src]
            w0[32:48, lo:hi] = kf[:, src]
            w0[49, lo:hi] = k0[src]
        if gi < 2:
            wfh[48, lo:hi] = bxd[src] + bhd[src] + dbd * k0[src]
            wfh[0:32, lo:hi] = rkd[:, src] + dwk0[:, src]
            w0[48, lo:hi] = bxd[src] + bhd[src]
            w0[0:32, lo:hi] = rkd[:, src]
        elif gi == 2:
            wfh[48, lo:hi] = bxd[src] + dbd * k0[src]
            wfh[0:32, lo:hi] = dwk0[:, src]
            w0[48, lo:hi] = bxd[src]
        else:
            wfh[48, lo:hi] = bhd[src]
            wfh[0:32, lo:hi] = rkd[:, src]
            w0[48, lo:hi] = bhd[src]
            w0[0:32, lo:hi] = rkd[:, src]

    wd4 = np.zeros((128, 4), np.float64)
    for c in range(SGC):
        wd4[32 * c:32 * c + 32, c] = dwd
    return wfh, w0, wd4


def _block_diag(w):
    """[50,128] fold -> per-gate block-diag (wh [4,128,128], wf [4,72,128])."""
    wh = np.zeros((4, 128, 128), np.float64)
    wf = np.zeros((4, 4 * FR, 128), np.float64)
    for g in range(4):
        cols = slice(32 * g, 32 * g + 32)
        for c in range(SGC):
            wh[g, 32 * c:32 * c + 32, 32 * c:32 * c + 32] = w[0:32, cols]
            wf[g, FR * c:FR * c + FR, 32 * c:32 * c + 32] = w[32:50, cols]
    return wh, wf


def _build_module(n_steps=T, debug=False):
    import concourse.bacc as bacc
    import concourse.mybir as mybir
    import concourse.tile as tile
    from contextlib import ExitStack

    f32 = mybir.dt.float32
    f16 = mybir.dt.float16
    AF = mybir.ActivationFunctionType

    nblk = (n_steps + 3) // 4

    nc = bacc.Bacc("TRN2")
    if debug:
        dbg_zr = nc.dram_tensor("dbg_zr", [128, 2 * CK], f32,
                                kind="ExternalOutput")
        dbg_xm = nc.dram_tensor("dbg_xm", [128, 2 * CK], f32,
                                kind="ExternalOutput")
        dbg_hs = nc.dram_tensor("dbg_hs", [128, CK], f16,
                                kind="ExternalOutput")
    feat = nc.dram_tensor("feat", [n_steps, NSG, 4 * FR, CK], f16,
                          kind="ExternalInput")
    h0 = nc.dram_tensor("h0", [H, BS], f16, kind="ExternalInput")
    wh_d = nc.dram_tensor("wh", [4, 128, 128], f16, kind="ExternalInput")
    wf_d = nc.dram_tensor("wf", [4, 4 * FR, 128], f16, kind="ExternalInput")
    wh0_d = nc.dram_tensor("wh0", [4, 128, 128], f16, kind="ExternalInput")
    wf0_d = nc.dram_tensor("wf0", [4, 4 * FR, 128], f16, kind="ExternalInput")
    wd_d = nc.dram_tensor("wd", [128, 4], f16, kind="ExternalInput")
    out = nc.dram_tensor("out", [n_steps, NSG, 4, CK], f32,
                         kind="ExternalOutput")

    with tile.TileContext(nc) as tc, ExitStack() as ctx:
        wpool = ctx.enter_context(tc.tile_pool(name="weights", bufs=1))
        fpool = ctx.enter_context(tc.tile_pool(name="feat", bufs=1))
        hpool = ctx.enter_context(tc.tile_pool(name="hs", bufs=1))
        ew = ctx.enter_context(tc.tile_pool(name="ew", bufs=2))
        ppool = ctx.enter_context(tc.tile_pool(name="psum", bufs=1, space="PSUM"))

        wh_s = [wpool.tile([128, 128], f16, tag=f"wh{g}", name=f"wh{g}")
                for g in range(4)]
        wf_s = [wpool.tile([4 * FR, 128], f16, tag=f"wf{g}", name=f"wf{g}")
                for g in range(4)]
        wh0_s = [wpool.tile([128, 128], f16, tag=f"wh0{g}", name=f"wh0{g}")
                 for g in range(4)]
        wf0_s = [wpool.tile([4 * FR, 128], f16, tag=f"wf0{g}", name=f"wf0{g}")
                 for g in range(4)]
        wd_s = wpool.tile([128, 4], f16, tag="wd")
        for g in range(4):
            nc.sync.dma_start(wh_s[g][:, :], wh_d[g, :, :])
            nc.sync.dma_start(wf_s[g][:, :], wf_d[g, :, :])
            nc.sync.dma_start(wh0_s[g][:, :], wh0_d[g, :, :])
            nc.sync.dma_start(wf0_s[g][:, :], wf0_d[g, :, :])
        nc.sync.dma_start(wd_s[:, :], wd_d[:, :])

        # Stacked state tiles [128, 512]: partitions = 4 chunks x 32 h
        h_s = []
        for g in range(NSG):
            t_ = hpool.tile([128, CK], f16, tag=f"hs{g}", name=f"hs{g}")
            for ci in range(SGC):
                c = g * SGC + ci
                nc.sync.dma_start(t_[32 * ci:32 * ci + 32, :],
                                  h0[:, c * CK:(c + 1) * CK])
            h_s.append(t_)

        # Feat tiles [72, 4*CK] double-buffered per sg; block b covers steps
        # 4b..4b+3 (slot = t%4).
        xf = [[fpool.tile([4 * FR, 4 * CK], f16, tag=f"xf{g}_{b}",
                          name=f"xf{g}_{b}") for b in range(2)]
              for g in range(NSG)]

        def feat_load(blk):
            t0 = 4 * blk
            t1 = min(t0 + 4, n_steps)
            for g in range(NSG):
                nc.sync.dma_start(
                    xf[g][blk % 2][:, 0:(t1 - t0) * CK].rearrange(
                        "r (s b) -> r s b", s=t1 - t0),
                    feat[t0:t1, g, :, :].rearrange("s r b -> r s b"))

        feat_load(0)
        if nblk > 1:
            feat_load(1)

        zr_ps = [ppool.tile([128, 2 * CK], f32, tag=f"zr{g}", name=f"zr{g}")
                 for g in range(NSG)]
        xm_ps = [ppool.tile([128, 2 * CK], f32, tag=f"xm{g}", name=f"xm{g}")
                 for g in range(NSG)]

        # Dense head for step td (reads h_s == h'(td)): both sgs write rows
        # 0:4 of zr_ps[0] (freed once sig consumed it), one ACT copy into the
        # 4-step fp32 accumulator, DMA per 4 steps.  Called from step td+1's
        # emission so the PE never stalls on td's elementwise chain.
        ysb = [None]

        def dense_head(td):
            slot = td % 4
            for g in range(NSG):
                nc.tensor.matmul(zr_ps[0][0:4, g * CK:(g + 1) * CK],
                                 lhsT=wd_s[:, :], rhs=h_s[g][:, :],
                                 start=True, stop=True)
            if slot == 0:
                ysb[0] = ew.tile([4, 8 * CK], f32, tag="ysb", name=f"ysb_{td}")
            nc.scalar.copy(ysb[0][:, slot * 2 * CK:(slot + 1) * 2 * CK],
                           zr_ps[0][0:4, :])
            if slot == 3 or td == n_steps - 1:
                t0 = td - slot
                nr = slot + 1
                nc.sync.dma_start(
                    out[t0:td + 1, :, :, :].rearrange("s g c b -> c s g b"),
                    ysb[0][:, 0:nr * 2 * CK].rearrange(
                        "c (s g b) -> c s g b", s=nr, g=NSG))

        for t in range(n_steps):
            blk, slot = t // 4, t % 4
            whg = wh0_s if t == 0 else wh_s
            wfg = wf0_s if t == 0 else wf_s

            # Phase 1a: previous step's dense head.  It reads h'(t-1) (the
            # same dependency the recurrent matmuls below have) and must
            # fully clear zr_ps[0] rows 0:4 (matmul + evac copy) before this
            # step's zr_ps[0] accumulation group opens.
            if t > 0:
                dense_head(t - 1)
            # Phase 1b: all feat matmuls (open the accumulation groups).
            # zr_ps[0]'s two go LAST so the PE has six independent matmuls
            # to chew on while the dense evac copy drains (WAR on bank 0).
            for gate, g in ((0, 1), (1, 1), (3, 0), (3, 1), (2, 0), (2, 1),
                            (0, 0), (1, 0)):
                bank, co = ((zr_ps[g], 0) if gate == 0 else
                            (zr_ps[g], CK) if gate == 1 else
                            (xm_ps[g], 0) if gate == 2 else (xm_ps[g], CK))
                rhs_f = xf[g][blk % 2][:, slot * CK:(slot + 1) * CK]
                nc.tensor.matmul(bank[:, co:co + CK],
                                 lhsT=wfg[gate][:, :], rhs=rhs_f,
                                 start=True, stop=False)
            # Phase 1c: recurrent matmuls (block-diag, read stacked state
            # h'(t-1) directly).  z,r first (sig), then mh (t2), then xh.
            for gate, bank_of, co in ((0, zr_ps, 0), (1, zr_ps, CK),
                                      (3, xm_ps, CK), (2, xm_ps, 0)):
                for g in range(NSG):
                    nc.tensor.matmul(bank_of[g][:, co:co + CK],
                                     lhsT=whg[gate][:, :], rhs=h_s[g][:, :],
                                     start=False, stop=True)

            # Feat prefetch for block blk+2.
            if slot == 3 and blk + 2 < nblk:
                feat_load(blk + 2)

            if debug and t == 0:
                dzr = ew.tile([128, 2 * CK], f32, tag="dzr", name="dzr")
                dxm = ew.tile([128, 2 * CK], f32, tag="dxm", name="dxm")
                nc.scalar.copy(dzr[:, :], zr_ps[0][:, :])
                nc.scalar.copy(dxm[:, :], xm_ps[0][:, :])
                nc.sync.dma_start(dbg_zr[:, :], dzr[:, :])
                nc.sync.dma_start(dbg_xm[:, :], dxm[:, :])

            # Phase 2: elementwise, interleaved across sgs.
            ewt = {}
            for g in range(NSG):
                ewt[g] = (
                    ew.tile([128, 2 * CK], f16, tag=f"zrs{g}", name=f"zrs{g}_{t}"),
                    ew.tile([128, CK], f16, tag=f"t2s{g}", name=f"t2s{g}_{t}"),
                    ew.tile([128, CK], f16, tag=f"t3s{g}", name=f"t3s{g}_{t}"),
                    ew.tile([128, CK], f16, tag=f"hhs{g}", name=f"hhs{g}_{t}"),
                    ew.tile([128, CK], f16, tag=f"ds{g}", name=f"ds{g}_{t}"),
                    ew.tile([128, CK], f16, tag=f"ps{g}", name=f"ps{g}_{t}"),
                )
            for g in range(NSG):
                zrs, t2, t3, hh, d, p = ewt[g]
                nc.scalar.activation(zrs[:, :], zr_ps[g][:, :], AF.Sigmoid)
                nc.vector.tensor_mul(t2[:, :], zrs[:, CK:2 * CK],
                                     xm_ps[g][:, CK:2 * CK])
                nc.vector.tensor_add(t3[:, :], t2[:, :], xm_ps[g][:, 0:CK])
                nc.scalar.activation(hh[:, :], t3[:, :], AF.Tanh)
                nc.vector.tensor_sub(d[:, :], h_s[g][:, :], hh[:, :])
                nc.vector.tensor_mul(p[:, :], zrs[:, 0:CK], d[:, :])
                nc.vector.tensor_add(h_s[g][:, :], hh[:, :], p[:, :])

            if debug and t == 0:
                nc.sync.dma_start(dbg_hs[:, :], h_s[0][:, :])
        dense_head(n_steps - 1)
    nc.compile()
    return nc


def _host_prep(inputs, n_steps=T):
    """Shard + transpose inputs host-side. Returns (in_maps, dense_b)."""
    dfeat = np.asarray(inputs["decoder_feature"], np.float32)
    y0 = np.asarray(inputs["decoder_init_input"], np.float32)
    h0 = np.asarray(inputs["init_state"], np.float32)
    wfh, w0, wd4 = _prep_weights_f64(
        np.asarray(inputs["kernel"], np.float32),
        np.asarray(inputs["recurrent_kernel"], np.float32),
        np.asarray(inputs["bias_x"], np.float32),
        np.asarray(inputs["bias_h"], np.float32),
        np.asarray(inputs["dense_w"], np.float32),
        np.asarray(inputs["dense_b"], np.float32),
    )
    wh, wf = _block_diag(wfh)
    wh0, wf0 = _block_diag(w0)
    wh = wh.astype(np.float16)
    wf = wf.astype(np.float16)
    wh0 = wh0.astype(np.float16)
    wf0 = wf0.astype(np.float16)
    wd = np.ascontiguousarray(wd4).astype(np.float16)

    def one(sl):
        nb = sl.stop - sl.start
        d = dfeat[sl, :n_steps]                       # [nb, T, F]
        d = d.reshape(NSG, SGC, CK, n_steps, F)
        fx = np.zeros((n_steps, NSG, SGC, FR, CK), np.float32)
        fx[:, :, :, 0:F, :] = d.transpose(3, 0, 1, 4, 2)
        fx[:, :, :, F, :] = 1.0                       # ones/bias plane
        fx[0, :, :, F + 1, :] = y0[sl, 0].reshape(NSG, SGC, CK)
        return {
            "feat": np.ascontiguousarray(
                fx.reshape(n_steps, NSG, 4 * FR, CK)).astype(np.float16),
            "h0": np.ascontiguousarray(h0[sl].T).astype(np.float16),
            "wh": wh, "wf": wf, "wh0": wh0, "wf0": wf0, "wd": wd,
        }

    in_maps = [one(slice(i * BS, (i + 1) * BS)) for i in range(NCORES)]
    return in_maps, float(np.asarray(inputs["dense_b"], np.float64)[0])


def run(inputs, trace=False, n_steps=T, debug=False, **spmd_kwargs):
    """Run on the 8 NeuronCores; returns (out [B,T,1] fp32, BassKernelResults)."""
    from concourse.bass_utils import run_bass_kernel_spmd

    key = (n_steps, debug)
    if key not in _CACHE:
        _CACHE[key] = _build_module(n_steps, debug=debug)
    nc = _CACHE[key]
    in_maps, db = _host_prep(inputs, n_steps)
    res = run_bass_kernel_spmd(nc, in_maps, list(range(NCORES)),
                               trace=trace, **spmd_kwargs)
    outs = []
    for r in res.results:
        o = np.asarray(r["out"])                      # [T, 2, 4, CK]
        outs.append(o.transpose(1, 2, 3, 0).reshape(BS, n_steps))
    full = np.concatenate(outs, axis=0)               # [B, T]
    return (full[:, :, None] + np.float32(db)).astype(np.float32), res


def kernel(**inputs) -> np.ndarray:
    out, _ = run(inputs, trace=False)
    return out


# revision 11
# speedup vs baseline: 1.1873x; 1.1873x over previous
"""GRU decoder (nn_Decoder2) Trainium2 Bass kernel — block-diagonal edition.

Per core (pure batch data-parallel over 8 cores): 4096 rows as 2 supergroups
of 4 chunks x 512.  State h lives STACKED: h_s[g] [128, 512] fp16 with
partitions = 4 chunks x 32 hidden units.

Key idea vs the 572us baseline: instead of 32 small M=32 matmuls per step
(one per gate x chunk, each streaming its own 512 rhs columns), each gate is
ONE K=128, M=128, N=512 matmul whose weights are the 32x32 recurrent block
replicated on the diagonal (block (ci,ci) = R'[32,32] at cols 32ci).  The
matmul then reads the stacked state tile DIRECTLY as rhs and produces the
gate for all 4 chunks in a single 512-column stream:
    out[32c+k, b] = sum_h R'[h,k] * h_s[32c+h, b].
The feature/bias/y0 side is a second accumulating matmul with block-diagonal
[72,128] weights (4 x 18 rows: 16 feat + ones + y0) reading a prefetched
feat tile [72, 512].  8 gate-MM pairs + 2 dense MMs per step, ~240ns each
(measured: full-M matmuls stream warm at ~N/2.4GHz with LDWEIGHTS hidden),
vs ~610ns cold-issue per baseline MM.  This also deletes the per-step
h'-scatter DMAs (matmul reads h_s in place).

PSUM: per sg, zr [128,1024] (z|r banks) and xm [128,1024] (xh|mh) = 8 banks.
Elementwise per sg, all ops [128, 512+] wide, fp16:
    zrs = sig(zr)  [128,1024];  t2 = zrs_r * mh;  t3 = t2 + xh;
    hh = tanh(t3);  d = h - hh;  p = zrs_z * d;  h' = hh + p.
Dense(1) folded into recurrent weights host-side (as baseline), so the
recurrence never waits on the dense output; per-step dense is a K=128 block
diag matmul into the freed xh bank rows 0:4, ACT-copied into a 4-step
accumulator and DMA'd out once per 4 steps.
"""
import numpy as np

B, T, F, H = 32768, 48, 16, 32
NCORES = 8
BS = B // NCORES            # 4096 batch per core
CK = 512                    # chunk batch size
NSG = 2                     # supergroups
SGC = 4                     # chunks per supergroup
FR = 18                     # feat rows per chunk: 16 feat + ones + y0

_CACHE = {}


def _prep_weights_f64(kernel, recurrent_kernel, bias_x, bias_h, dense_w, dense_b):
    """Baseline fold (float64): returns wfh [50,128], w0 [50,128], wd4 [128,4].

    Gate column blocks: 0:32 z, 32:64 r, 64:96 xh, 96:128 mh.
    Row layout: 0:32 h, 32:48 feat, 48 ones/bias, 49 y0 (zero in wfh).
    """
    kd = kernel.astype(np.float64)
    rkd = recurrent_kernel.astype(np.float64)
    bxd = bias_x.astype(np.float64)
    bhd = bias_h.astype(np.float64)
    dwd = dense_w.astype(np.float64)[:, 0]
    dbd = float(dense_b.astype(np.float64)[0])

    k0 = kd[0]
    kf = kd[1:]
    dwk0 = np.outer(dwd, k0)

    wfh = np.zeros((50, 128), np.float64)
    w0 = np.zeros((50, 128), np.float64)
    for gi, (lo, hi) in enumerate([(0, 32), (32, 64), (64, 96), (96, 128)]):
        src = [slice(0, 32), slice(32, 64), slice(64, 96), slice(64, 96)][gi]
        if gi < 3:
            wfh[32:48, lo:hi] = kf[:, src]
            w0[32:48, lo:hi] = kf[:, src]
            w0[49, lo:hi] = k0[src]
        if gi < 2:
            wfh[48, lo:hi] = bxd[src] + bhd[src] + dbd * k0[src]
            wfh[0:32, lo:hi] = rkd[:, src] + dwk0[:, src]
            w0[48, lo:hi] = bxd[src] + bhd[src]
            w0[0:32, lo:hi] = rkd[:, src]
        elif gi == 2:
            wfh[48, lo:hi] = bxd[src] + dbd * k0[src]
            wfh[0:32, lo:hi] = dwk0[:, src]
            w0[48, lo:hi] = bxd[src]
        else:
            wfh[48, lo:hi] = bhd[src]
            wfh[0:32, lo:hi] = rkd[:, src]
            w0[48, lo:hi] = bhd[src]
            w0[0:32, lo:hi] = rkd[:, src]

    wd4 = np.zeros((128, 4), np.float64)
    for c in range(SGC):
        wd4[32 * c:32 * c + 32, c] = dwd
    return wfh, w0, wd4


def _block_diag(w):
    """[50,128] fold -> per-gate block-diag (wh [4,128,128], wf [4,72,128])."""
    wh = np.zeros((4, 128, 128), np.float64)
    wf = np.zeros((4, 4 * FR, 128), np.float64)
    for g in range(4):
        cols = slice(32 * g, 32 * g + 32)
        for c in range(SGC):
            wh[g, 32 * c:32 * c + 32, 32 * c:32 * c + 32] = w[0:32, cols]
            wf[g, FR * c:FR * c + FR, 32 * c:32 * c + 32] = w[32:50, cols]
    return wh, wf


def _build_module(n_steps=T, debug=False):
    import concourse.bacc as bacc
    import concourse.mybir as mybir
    import concourse.tile as tile
    from contextlib import ExitStack

    f32 = mybir.dt.float32
    f16 = mybir.dt.float16
    AF = mybir.ActivationFunctionType

    nblk = (n_steps + 3) // 4

    nc = bacc.Bacc("TRN2")
    if debug:
        dbg_zr = nc.dram_tensor("dbg_zr", [128, 2 * CK], f32,
                                kind="ExternalOutput")
        dbg_xm = nc.dram_tensor("dbg_xm", [128, 2 * CK], f32,
                                kind="ExternalOutput")
        dbg_hs = nc.dram_tensor("dbg_hs", [128, CK], f16,
                                kind="ExternalOutput")
    feat = nc.dram_tensor("feat", [n_steps, NSG, 4 * FR, CK], f16,
                          kind="ExternalInput")
    h0 = nc.dram_tensor("h0", [H, BS], f16, kind="ExternalInput")
    wh_d = nc.dram_tensor("wh", [4, 128, 128], f16, kind="ExternalInput")
    wf_d = nc.dram_tensor("wf", [4, 4 * FR, 128], f16, kind="ExternalInput")
    wh0_d = nc.dram_tensor("wh0", [4, 128, 128], f16, kind="ExternalInput")
    wf0_d = nc.dram_tensor("wf0", [4, 4 * FR, 128], f16, kind="ExternalInput")
    wd_d = nc.dram_tensor("wd", [128, 4], f16, kind="ExternalInput")
    out = nc.dram_tensor("out", [n_steps, NSG, 4, CK], f32,
                         kind="ExternalOutput")

    with tile.TileContext(nc) as tc, ExitStack() as ctx:
        wpool = ctx.enter_context(tc.tile_pool(name="weights", bufs=1))
        fpool = ctx.enter_context(tc.tile_pool(name="feat", bufs=1))
        hpool = ctx.enter_context(tc.tile_pool(name="hs", bufs=1))
        ew = ctx.enter_context(tc.tile_pool(name="ew", bufs=2))
        ppool = ctx.enter_context(tc.tile_pool(name="psum", bufs=1, space="PSUM"))

        wh_s = [wpool.tile([128, 128], f16, tag=f"wh{g}", name=f"wh{g}")
                for g in range(4)]
        wf_s = [wpool.tile([4 * FR, 128], f16, tag=f"wf{g}", name=f"wf{g}")
                for g in range(4)]
        wh0_s = [wpool.tile([128, 128], f16, tag=f"wh0{g}", name=f"wh0{g}")
                 for g in range(4)]
        wf0_s = [wpool.tile([4 * FR, 128], f16, tag=f"wf0{g}", name=f"wf0{g}")
                 for g in range(4)]
        wd_s = wpool.tile([128, 4], f16, tag="wd")
        for g in range(4):
            nc.sync.dma_start(wh_s[g][:, :], wh_d[g, :, :])
            nc.sync.dma_start(wf_s[g][:, :], wf_d[g, :, :])
            nc.sync.dma_start(wh0_s[g][:, :], wh0_d[g, :, :])
            nc.sync.dma_start(wf0_s[g][:, :], wf0_d[g, :, :])
        nc.sync.dma_start(wd_s[:, :], wd_d[:, :])

        # Stacked state tiles [128, 512]: partitions = 4 chunks x 32 h
        h_s = []
        for g in range(NSG):
            t_ = hpool.tile([128, CK], f16, tag=f"hs{g}", name=f"hs{g}")
            for ci in range(SGC):
                c = g * SGC + ci
                nc.sync.dma_start(t_[32 * ci:32 * ci + 32, :],
                                  h0[:, c * CK:(c + 1) * CK])
            h_s.append(t_)

        # Feat tiles [72, 4*CK] double-buffered per sg; block b covers steps
        # 4b..4b+3 (slot = t%4).
        xf = [[fpool.tile([4 * FR, 4 * CK], f16, tag=f"xf{g}_{b}",
                          name=f"xf{g}_{b}") for b in range(2)]
              for g in range(NSG)]

        def feat_load(blk):
            t0 = 4 * blk
            t1 = min(t0 + 4, n_steps)
            for g in range(NSG):
                nc.sync.dma_start(
                    xf[g][blk % 2][:, 0:(t1 - t0) * CK].rearrange(
                        "r (s b) -> r s b", s=t1 - t0),
                    feat[t0:t1, g, :, :].rearrange("s r b -> r s b"))

        feat_load(0)
        if nblk > 1:
            feat_load(1)

        zr_ps = [ppool.tile([128, 2 * CK], f32, tag=f"zr{g}", name=f"zr{g}")
                 for g in range(NSG)]
        xm_ps = [ppool.tile([128, 2 * CK], f32, tag=f"xm{g}", name=f"xm{g}")
                 for g in range(NSG)]

        # Dense head for step td (reads h_s == h'(td)): both sgs write rows
        # 0:4 of zr_ps[0] (freed once sig consumed it), one ACT copy into the
        # 4-step fp32 accumulator, DMA per 4 steps.  Called from step td+1's
        # emission so the PE never stalls on td's elementwise chain.
        ysb = [None]

        def dense_head(td):
            slot = td % 4
            for g in range(NSG):
                nc.tensor.matmul(zr_ps[0][0:4, g * CK:(g + 1) * CK],
                                 lhsT=wd_s[:, :], rhs=h_s[g][:, :],
                                 start=True, stop=True)
            if slot == 0:
                ysb[0] = ew.tile([4, 8 * CK], f32, tag="ysb", name=f"ysb_{td}")
            nc.scalar.copy(ysb[0][:, slot * 2 * CK:(slot + 1) * 2 * CK],
                           zr_ps[0][0:4, :])
            if slot == 3 or td == n_steps - 1:
                t0 = td - slot
                nr = slot + 1
                nc.sync.dma_start(
                    out[t0:td + 1, :, :, :].rearrange("s g c b -> c s g b"),
                    ysb[0][:, 0:nr * 2 * CK].rearrange(
                        "c (s g b) -> c s g b", s=nr, g=NSG))

        # Emission is organized in per-supergroup UNITS so each engine's
        # FIFO alternates whole dependency chains: U(0,t) then U(1,t).
        # Interleaving sgs op-by-op couples the chains through the strict
        # per-engine FIFOs (sg0's tanh queued behind sg1's sig, which waits
        # on sg1's matmuls...), doubling the effective step period.
        for t in range(n_steps):
            blk, slot = t // 4, t % 4
            whg = wh0_s if t == 0 else wh_s
            wfg = wf0_s if t == 0 else wf_s
            gate_layout = ((0, zr_ps, 0), (1, zr_ps, CK),
                           (3, xm_ps, CK), (2, xm_ps, 0))

            for g in range(NSG):
                # --- unit U(g, t) ---
                if g == 0 and t > 0:
                    # Dense head of t-1 (both sgs): same h'(t-1) dependency
                    # as this unit's recurrent matmuls; must clear
                    # zr_ps[0][0:4] before the new zr accumulation opens.
                    dense_head(t - 1)
                # Feat matmuls: open accumulation groups (no h' dependency;
                # PE lead work while the previous chain drains).
                for gate, bank_of, co in gate_layout:
                    rhs_f = xf[g][blk % 2][:, slot * CK:(slot + 1) * CK]
                    nc.tensor.matmul(bank_of[g][:, co:co + CK],
                                     lhsT=wfg[gate][:, :], rhs=rhs_f,
                                     start=True, stop=False)
                # Recurrent matmuls (block-diag, read stacked h'(t-1)):
                # z,r first (sig), then mh (t2), then xh (t3).
                for gate, bank_of, co in gate_layout:
                    nc.tensor.matmul(bank_of[g][:, co:co + CK],
                                     lhsT=whg[gate][:, :], rhs=h_s[g][:, :],
                                     start=False, stop=True)
                # Elementwise chain of this sg.
                zrs = ew.tile([128, 2 * CK], f16, tag=f"zrs{g}",
                              name=f"zrs{g}_{t}")
                t2 = ew.tile([128, CK], f16, tag=f"t2s{g}", name=f"t2s{g}_{t}")
                t3 = ew.tile([128, CK], f16, tag=f"t3s{g}", name=f"t3s{g}_{t}")
                hh = ew.tile([128, CK], f16, tag=f"hhs{g}", name=f"hhs{g}_{t}")
                d = ew.tile([128, CK], f16, tag=f"ds{g}", name=f"ds{g}_{t}")
                p = ew.tile([128, CK], f16, tag=f"ps{g}", name=f"ps{g}_{t}")
                nc.scalar.activation(zrs[:, :], zr_ps[g][:, :], AF.Sigmoid)
                nc.vector.tensor_mul(t2[:, :], zrs[:, CK:2 * CK],
                                     xm_ps[g][:, CK:2 * CK])
                nc.vector.tensor_add(t3[:, :], t2[:, :], xm_ps[g][:, 0:CK])
                nc.scalar.activation(hh[:, :], t3[:, :], AF.Tanh)
                nc.vector.tensor_sub(d[:, :], h_s[g][:, :], hh[:, :])
                nc.vector.tensor_mul(p[:, :], zrs[:, 0:CK], d[:, :])
                nc.vector.tensor_add(h_s[g][:, :], hh[:, :], p[:, :])
                if g == 1 and slot == 3 and blk + 2 < nblk:
                    feat_load(blk + 2)

            if debug and t == 0:
                dzr = ew.tile([128, 2 * CK], f32, tag="dzr", name="dzr")
                dxm = ew.tile([128, 2 * CK], f32, tag="dxm", name="dxm")
                nc.scalar.copy(dzr[:, :], zr_ps[0][:, :])
                nc.scalar.copy(dxm[:, :], xm_ps[0][:, :])
                nc.sync.dma_start(dbg_zr[:, :], dzr[:, :])
                nc.sync.dma_start(dbg_xm[:, :], dxm[:, :])
                nc.sync.dma_start(dbg_hs[:, :], h_s[0][:, :])
        dense_head(n_steps - 1)
    nc.compile()
    return nc


def _host_prep(inputs, n_steps=T):
    """Shard + transpose inputs host-side. Returns (in_maps, dense_b)."""
    dfeat = np.asarray(inputs["decoder_feature"], np.float32)
    y0 = np.asarray(inputs["decoder_init_input"], np.float32)
    h0 = np.asarray(inputs["init_state"], np.float32)
    wfh, w0, wd4 = _prep_weights_f64(
        np.asarray(inputs["kernel"], np.float32),
        np.asarray(inputs["recurrent_kernel"], np.float32),
        np.asarray(inputs["bias_x"], np.float32),
        np.asarray(inputs["bias_h"], np.float32),
        np.asarray(inputs["dense_w"], np.float32),
        np.asarray(inputs["dense_b"], np.float32),
    )
    wh, wf = _block_diag(wfh)
    wh0, wf0 = _block_diag(w0)
    wh = wh.astype(np.float16)
    wf = wf.astype(np.float16)
    wh0 = wh0.astype(np.float16)
    wf0 = wf0.astype(np.float16)
    wd = np.ascontiguousarray(wd4).astype(np.float16)

    def one(sl):
        nb = sl.stop - sl.start
        d = dfeat[sl, :n_steps]                       # [nb, T, F]
        d = d.reshape(NSG, SGC, CK, n_steps, F)
        fx = np.zeros((n_steps, NSG, SGC, FR, CK), np.float32)
        fx[:, :, :, 0:F, :] = d.transpose(3, 0, 1, 4, 2)
        fx[:, :, :, F, :] = 1.0                       # ones/bias plane
        fx[0, :, :, F + 1, :] = y0[sl, 0].reshape(NSG, SGC, CK)
        return {
            "feat": np.ascontiguousarray(
                fx.reshape(n_steps, NSG, 4 * FR, CK)).astype(np.float16),
            "h0": np.ascontiguousarray(h0[sl].T).astype(np.float16),
            "wh": wh, "wf": wf, "wh0": wh0, "wf0": wf0, "wd": wd,
        }

    in_maps = [one(slice(i * BS, (i + 1) * BS)) for i in range(NCORES)]
    return in_maps, float(np.asarray(inputs["dense_b"], np.float64)[0])


def run(inputs, trace=False, n_steps=T, debug=False, **spmd_kwargs):
    """Run on the 8 NeuronCores; returns (out [B,T,1] fp32, BassKernelResults)."""
    from concourse.bass_utils import run_bass_kernel_spmd

    key = (n_steps, debug)
    if key not in _CACHE:
        _CACHE[key] = _build_module(n_steps, debug=debug)
    nc = _CACHE[key]
    in_maps, db = _host_prep(inputs, n_steps)
    res = run_bass_kernel_spmd(nc, in_maps, list(range(NCORES)),
                               trace=trace, **spmd_kwargs)
    outs = []
    for r in res.results:
        o = np.asarray(r["out"])                      # [T, 2, 4, CK]
        outs.append(o.transpose(1, 2, 3, 0).reshape(BS, n_steps))
    full = np.concatenate(outs, axis=0)               # [B, T]
    return (full[:, :, None] + np.float32(db)).astype(np.float32), res


def kernel(**inputs) -> np.ndarray:
    out, _ = run(inputs, trace=False)
    return out


# revision 12
# speedup vs baseline: 1.3173x; 1.1095x over previous
"""GRU decoder (nn_Decoder2) Trainium2 Bass kernel — block-diagonal edition.

Per core (pure batch data-parallel over 8 cores): 4096 rows as 2 supergroups
of 4 chunks x 512.  State h lives STACKED: h_s[g] [128, 512] fp16 with
partitions = 4 chunks x 32 hidden units.

Key idea vs the 572us baseline: instead of 32 small M=32 matmuls per step
(one per gate x chunk, each streaming its own 512 rhs columns), each gate is
ONE K=128, M=128, N=512 matmul whose weights are the 32x32 recurrent block
replicated on the diagonal (block (ci,ci) = R'[32,32] at cols 32ci).  The
matmul then reads the stacked state tile DIRECTLY as rhs and produces the
gate for all 4 chunks in a single 512-column stream:
    out[32c+k, b] = sum_h R'[h,k] * h_s[32c+h, b].
The feature/bias/y0 side is a second accumulating matmul with block-diagonal
[72,128] weights (4 x 18 rows: 16 feat + ones + y0) reading a prefetched
feat tile [72, 512].  8 gate-MM pairs + 2 dense MMs per step, ~240ns each
(measured: full-M matmuls stream warm at ~N/2.4GHz with LDWEIGHTS hidden),
vs ~610ns cold-issue per baseline MM.  This also deletes the per-step
h'-scatter DMAs (matmul reads h_s in place).

PSUM: per sg, zr [128,1024] (z|r banks) and xm [128,1024] (xh|mh) = 8 banks.
Elementwise per sg, all ops [128, 512+] wide, fp16:
    zrs = sig(zr)  [128,1024];  t2 = zrs_r * mh;  t3 = t2 + xh;
    hh = tanh(t3);  d = h - hh;  p = zrs_z * d;  h' = hh + p.
Dense(1) folded into recurrent weights host-side (as baseline), so the
recurrence never waits on the dense output; per-step dense is a K=128 block
diag matmul into the freed xh bank rows 0:4, ACT-copied into a 4-step
accumulator and DMA'd out once per 4 steps.
"""
import numpy as np

B, T, F, H = 32768, 48, 16, 32
NCORES = 8
BS = B // NCORES            # 4096 batch per core
CK = 512                    # chunk batch size
NSG = 2                     # supergroups
SGC = 4                     # chunks per supergroup
FR = 18                     # feat rows per chunk: 16 feat + ones + y0

_CACHE = {}


def _prep_weights_f64(kernel, recurrent_kernel, bias_x, bias_h, dense_w, dense_b):
    """Baseline fold (float64): returns wfh [50,128], w0 [50,128], wd4 [128,4].

    Gate column blocks: 0:32 z, 32:64 r, 64:96 xh, 96:128 mh.
    Row layout: 0:32 h, 32:48 feat, 48 ones/bias, 49 y0 (zero in wfh).
    """
    kd = kernel.astype(np.float64)
    rkd = recurrent_kernel.astype(np.float64)
    bxd = bias_x.astype(np.float64)
    bhd = bias_h.astype(np.float64)
    dwd = dense_w.astype(np.float64)[:, 0]
    dbd = float(dense_b.astype(np.float64)[0])

    k0 = kd[0]
    kf = kd[1:]
    dwk0 = np.outer(dwd, k0)

    wfh = np.zeros((50, 128), np.float64)
    w0 = np.zeros((50, 128), np.float64)
    for gi, (lo, hi) in enumerate([(0, 32), (32, 64), (64, 96), (96, 128)]):
        src = [slice(0, 32), slice(32, 64), slice(64, 96), slice(64, 96)][gi]
        if gi < 3:
            wfh[32:48, lo:hi] = kf[:, src]
            w0[32:48, lo:hi] = kf[:, src]
            w0[49, lo:hi] = k0[src]
        if gi < 2:
            wfh[48, lo:hi] = bxd[src] + bhd[src] + dbd * k0[src]
            wfh[0:32, lo:hi] = rkd[:, src] + dwk0[:, src]
            w0[48, lo:hi] = bxd[src] + bhd[src]
            w0[0:32, lo:hi] = rkd[:, src]
        elif gi == 2:
            wfh[48, lo:hi] = bxd[src] + dbd * k0[src]
            wfh[0:32, lo:hi] = dwk0[:, src]
            w0[48, lo:hi] = bxd[src]
        else:
            wfh[48, lo:hi] = bhd[src]
            wfh[0:32, lo:hi] = rkd[:, src]
            w0[48, lo:hi] = bhd[src]
            w0[0:32, lo:hi] = rkd[:, src]

    wd4 = np.zeros((128, 4), np.float64)
    for c in range(SGC):
        wd4[32 * c:32 * c + 32, c] = dwd
    return wfh, w0, wd4


def _block_diag(w):
    """[50,128] fold -> per-gate block-diag (wh [4,128,128], wf [4,72,128])."""
    wh = np.zeros((4, 128, 128), np.float64)
    wf = np.zeros((4, 4 * FR, 128), np.float64)
    for g in range(4):
        cols = slice(32 * g, 32 * g + 32)
        for c in range(SGC):
            wh[g, 32 * c:32 * c + 32, 32 * c:32 * c + 32] = w[0:32, cols]
            wf[g, FR * c:FR * c + FR, 32 * c:32 * c + 32] = w[32:50, cols]
    return wh, wf


def _build_module(n_steps=T, debug=False):
    import concourse.bacc as bacc
    import concourse.mybir as mybir
    import concourse.tile as tile
    from contextlib import ExitStack

    f32 = mybir.dt.float32
    f16 = mybir.dt.float16
    AF = mybir.ActivationFunctionType

    nblk = (n_steps + 3) // 4

    nc = bacc.Bacc("TRN2")
    if debug:
        dbg_zr = nc.dram_tensor("dbg_zr", [128, 2 * CK], f32,
                                kind="ExternalOutput")
        dbg_xm = nc.dram_tensor("dbg_xm", [128, 2 * CK], f32,
                                kind="ExternalOutput")
        dbg_hs = nc.dram_tensor("dbg_hs", [128, CK], f16,
                                kind="ExternalOutput")
    feat = nc.dram_tensor("feat", [n_steps, NSG, 4 * FR, CK], f16,
                          kind="ExternalInput")
    h0 = nc.dram_tensor("h0", [H, BS], f16, kind="ExternalInput")
    wh_d = nc.dram_tensor("wh", [4, 128, 128], f16, kind="ExternalInput")
    wf_d = nc.dram_tensor("wf", [4, 4 * FR, 128], f16, kind="ExternalInput")
    wh0_d = nc.dram_tensor("wh0", [4, 128, 128], f16, kind="ExternalInput")
    wf0_d = nc.dram_tensor("wf0", [4, 4 * FR, 128], f16, kind="ExternalInput")
    wd_d = nc.dram_tensor("wd", [128, 4], f16, kind="ExternalInput")
    out = nc.dram_tensor("out", [n_steps, NSG, 4, CK], f32,
                         kind="ExternalOutput")

    with tile.TileContext(nc) as tc, ExitStack() as ctx:
        wpool = ctx.enter_context(tc.tile_pool(name="weights", bufs=1))
        fpool = ctx.enter_context(tc.tile_pool(name="feat", bufs=1))
        hpool = ctx.enter_context(tc.tile_pool(name="hs", bufs=1))
        ew = ctx.enter_context(tc.tile_pool(name="ew", bufs=2))
        ppool = ctx.enter_context(tc.tile_pool(name="psum", bufs=1, space="PSUM"))

        wh_s = [wpool.tile([128, 128], f16, tag=f"wh{g}", name=f"wh{g}")
                for g in range(4)]
        wf_s = [wpool.tile([4 * FR, 128], f16, tag=f"wf{g}", name=f"wf{g}")
                for g in range(4)]
        wh0_s = [wpool.tile([128, 128], f16, tag=f"wh0{g}", name=f"wh0{g}")
                 for g in range(4)]
        wf0_s = [wpool.tile([4 * FR, 128], f16, tag=f"wf0{g}", name=f"wf0{g}")
                 for g in range(4)]
        wd_s = wpool.tile([128, 4], f16, tag="wd")
        for g in range(4):
            nc.sync.dma_start(wh_s[g][:, :], wh_d[g, :, :])
            nc.sync.dma_start(wf_s[g][:, :], wf_d[g, :, :])
            nc.sync.dma_start(wh0_s[g][:, :], wh0_d[g, :, :])
            nc.sync.dma_start(wf0_s[g][:, :], wf0_d[g, :, :])
        nc.sync.dma_start(wd_s[:, :], wd_d[:, :])

        # Stacked state tiles [128, 512]: partitions = 4 chunks x 32 h
        h_s = []
        for g in range(NSG):
            t_ = hpool.tile([128, CK], f16, tag=f"hs{g}", name=f"hs{g}")
            for ci in range(SGC):
                c = g * SGC + ci
                nc.sync.dma_start(t_[32 * ci:32 * ci + 32, :],
                                  h0[:, c * CK:(c + 1) * CK])
            h_s.append(t_)

        # Feat tiles [72, 4*CK] double-buffered per sg; block b covers steps
        # 4b..4b+3 (slot = t%4).
        xf = [[fpool.tile([4 * FR, 4 * CK], f16, tag=f"xf{g}_{b}",
                          name=f"xf{g}_{b}") for b in range(2)]
              for g in range(NSG)]

        def feat_load(blk):
            t0 = 4 * blk
            t1 = min(t0 + 4, n_steps)
            for g in range(NSG):
                nc.sync.dma_start(
                    xf[g][blk % 2][:, 0:(t1 - t0) * CK].rearrange(
                        "r (s b) -> r s b", s=t1 - t0),
                    feat[t0:t1, g, :, :].rearrange("s r b -> r s b"))

        feat_load(0)
        if nblk > 1:
            feat_load(1)

        zr_ps = [ppool.tile([128, 2 * CK], f32, tag=f"zr{g}", name=f"zr{g}")
                 for g in range(NSG)]
        xm_ps = [ppool.tile([128, 2 * CK], f32, tag=f"xm{g}", name=f"xm{g}")
                 for g in range(NSG)]

        # Dense head for step td (reads h_s == h'(td)): both sgs write rows
        # 0:4 of zr_ps[0] (freed once sig consumed it), one ACT copy into the
        # 4-step fp32 accumulator, DMA per 4 steps.  Called from step td+1's
        # emission so the PE never stalls on td's elementwise chain.
        ysb = [None]

        def dense_head(td):
            slot = td % 4
            for g in range(NSG):
                nc.tensor.matmul(zr_ps[0][0:4, g * CK:(g + 1) * CK],
                                 lhsT=wd_s[:, :], rhs=h_s[g][:, :],
                                 start=True, stop=True)
            if slot == 0:
                ysb[0] = ew.tile([4, 8 * CK], f32, tag="ysb", name=f"ysb_{td}")
            nc.scalar.copy(ysb[0][:, slot * 2 * CK:(slot + 1) * 2 * CK],
                           zr_ps[0][0:4, :])
            if slot == 3 or td == n_steps - 1:
                t0 = td - slot
                nr = slot + 1
                nc.sync.dma_start(
                    out[t0:td + 1, :, :, :].rearrange("s g c b -> c s g b"),
                    ysb[0][:, 0:nr * 2 * CK].rearrange(
                        "c (s g b) -> c s g b", s=nr, g=NSG))

        # Emission is organized in per-supergroup UNITS so each engine's
        # FIFO alternates whole dependency chains: U(0,t) then U(1,t).
        # Interleaving sgs op-by-op couples the chains through the strict
        # per-engine FIFOs (sg0's tanh queued behind sg1's sig, which waits
        # on sg1's matmuls...), doubling the effective step period.
        for t in range(n_steps):
            blk, slot = t // 4, t % 4
            whg = wh0_s if t == 0 else wh_s
            wfg = wf0_s if t == 0 else wf_s
            gate_layout = ((0, zr_ps, 0), (1, zr_ps, CK),
                           (3, xm_ps, CK), (2, xm_ps, 0))

            for g in range(NSG):
                # --- unit U(g, t) ---
                if g == 0 and t > 0:
                    # Dense head of t-1 (both sgs): same h'(t-1) dependency
                    # as this unit's recurrent matmuls; must clear
                    # zr_ps[0][0:4] before the new zr accumulation opens.
                    dense_head(t - 1)
                # Feat matmuls: open accumulation groups (no h' dependency;
                # PE lead work while the previous chain drains).
                for gate, bank_of, co in gate_layout:
                    rhs_f = xf[g][blk % 2][:, slot * CK:(slot + 1) * CK]
                    nc.tensor.matmul(bank_of[g][:, co:co + CK],
                                     lhsT=wfg[gate][:, :], rhs=rhs_f,
                                     start=True, stop=False)
                # Recurrent matmuls (block-diag, read stacked h'(t-1)):
                # z,r first (sig), then mh (t2), then xh (t3).
                for gate, bank_of, co in gate_layout:
                    nc.tensor.matmul(bank_of[g][:, co:co + CK],
                                     lhsT=whg[gate][:, :], rhs=h_s[g][:, :],
                                     start=False, stop=True)
                # Elementwise chain of this sg.
                zrs = ew.tile([128, 2 * CK], f16, tag=f"zrs{g}",
                              name=f"zrs{g}_{t}")
                t2 = ew.tile([128, CK], f16, tag=f"t2s{g}", name=f"t2s{g}_{t}")
                t3 = ew.tile([128, CK], f16, tag=f"t3s{g}", name=f"t3s{g}_{t}")
                hh = ew.tile([128, CK], f16, tag=f"hhs{g}", name=f"hhs{g}_{t}")
                d = ew.tile([128, CK], f16, tag=f"ds{g}", name=f"ds{g}_{t}")
                p = ew.tile([128, CK], f16, tag=f"ps{g}", name=f"ps{g}_{t}")
                # Blend via h' = z*h - (z-1)*hh: ds = z*h runs OFF the tanh
                # critical path; only es2 (fused scalar_tensor_tensor) and
                # the final sub follow tanh -> 2 post-tanh hops, not 3.
                nc.scalar.activation(zrs[:, :], zr_ps[g][:, :], AF.Sigmoid)
                nc.vector.tensor_mul(t2[:, :], zrs[:, CK:2 * CK],
                                     xm_ps[g][:, CK:2 * CK])
                nc.vector.tensor_add(t3[:, :], t2[:, :], xm_ps[g][:, 0:CK])
                nc.vector.tensor_mul(d[:, :], zrs[:, 0:CK], h_s[g][:, :])
                nc.scalar.activation(hh[:, :], t3[:, :], AF.Tanh)
                nc.vector.scalar_tensor_tensor(
                    p[:, :], zrs[:, 0:CK], 1.0, hh[:, :],
                    mybir.AluOpType.subtract, mybir.AluOpType.mult)
                nc.vector.tensor_sub(h_s[g][:, :], d[:, :], p[:, :])
                if g == 1 and slot == 3 and blk + 2 < nblk:
                    feat_load(blk + 2)

            if debug and t == 0:
                dzr = ew.tile([128, 2 * CK], f32, tag="dzr", name="dzr")
                dxm = ew.tile([128, 2 * CK], f32, tag="dxm", name="dxm")
                nc.scalar.copy(dzr[:, :], zr_ps[0][:, :])
                nc.scalar.copy(dxm[:, :], xm_ps[0][:, :])
                nc.sync.dma_start(dbg_zr[:, :], dzr[:, :])
                nc.sync.dma_start(dbg_xm[:, :], dxm[:, :])
                nc.sync.dma_start(dbg_hs[:, :], h_s[0][:, :])
        dense_head(n_steps - 1)
    nc.compile()
    return nc


def _host_prep(inputs, n_steps=T):
    """Shard + transpose inputs host-side. Returns (in_maps, dense_b)."""
    dfeat = np.asarray(inputs["decoder_feature"], np.float32)
    y0 = np.asarray(inputs["decoder_init_input"], np.float32)
    h0 = np.asarray(inputs["init_state"], np.float32)
    wfh, w0, wd4 = _prep_weights_f64(
        np.asarray(inputs["kernel"], np.float32),
        np.asarray(inputs["recurrent_kernel"], np.float32),
        np.asarray(inputs["bias_x"], np.float32),
        np.asarray(inputs["bias_h"], np.float32),
        np.asarray(inputs["dense_w"], np.float32),
        np.asarray(inputs["dense_b"], np.float32),
    )
    wh, wf = _block_diag(wfh)
    wh0, wf0 = _block_diag(w0)
    wh = wh.astype(np.float16)
    wf = wf.astype(np.float16)
    wh0 = wh0.astype(np.float16)
    wf0 = wf0.astype(np.float16)
    wd = np.ascontiguousarray(wd4).astype(np.float16)

    def one(sl):
        nb = sl.stop - sl.start
        d = dfeat[sl, :n_steps]                       # [nb, T, F]
        d = d.reshape(NSG, SGC, CK, n_steps, F)
        fx = np.zeros((n_steps, NSG, SGC, FR, CK), np.float32)
        fx[:, :, :, 0:F, :] = d.transpose(3, 0, 1, 4, 2)
        fx[:, :, :, F, :] = 1.0                       # ones/bias plane
        fx[0, :, :, F + 1, :] = y0[sl, 0].reshape(NSG, SGC, CK)
        return {
            "feat": np.ascontiguousarray(
                fx.reshape(n_steps, NSG, 4 * FR, CK)).astype(np.float16),
            "h0": np.ascontiguousarray(h0[sl].T).astype(np.float16),
            "wh": wh, "wf": wf, "wh0": wh0, "wf0": wf0, "wd": wd,
        }

    in_maps = [one(slice(i * BS, (i + 1) * BS)) for i in range(NCORES)]
    return in_maps, float(np.asarray(inputs["dense_b"], np.float64)[0])


def run(inputs, trace=False, n_steps=T, debug=False, **spmd_kwargs):
    """Run on the 8 NeuronCores; returns (out [B,T,1] fp32, BassKernelResults)."""
    from concourse.bass_utils import run_bass_kernel_spmd

    key = (n_steps, debug)
    if key not in _CACHE:
        _CACHE[key] = _build_module(n_steps, debug=debug)
    nc = _CACHE[key]
    in_maps, db = _host_prep(inputs, n_steps)
    res = run_bass_kernel_spmd(nc, in_maps, list(range(NCORES)),
                               trace=trace, **spmd_kwargs)
    outs = []
    for r in res.results:
        o = np.asarray(r["out"])                      # [T, 2, 4, CK]
        outs.append(o.transpose(1, 2, 3, 0).reshape(BS, n_steps))
    full = np.concatenate(outs, axis=0)               # [B, T]
    return (full[:, :, None] + np.float32(db)).astype(np.float32), res


def kernel(**inputs) -> np.ndarray:
    out, _ = run(inputs, trace=False)
    return out
